# revision 15
# baseline (speedup 1.0000x reference)
"""Antialiased bicubic 4x downscale (blur -> bicubic/2, twice) on 8 TRN2 cores.

The whole chain is linear and separable: every stage is M_H (x) M_W acting on
the H/W axes, so the composition collapses to a single 1024->256 banded matrix
T applied on both sides: out = T @ X @ T^T per (batch, channel) image.

Sharding: pure data parallel - batch 16 -> 2 per core, 6 images/core.

Schedule (v3): the measured window starts right after the engine barrier and
ends after the fixed semaphore-sweep postamble; SWDGE with full-row 4 KB
descriptors sustains ~400 GB/s, so the kernel keeps one deep SWDGE stream and
gets everything else out of its way.

  x arrives in row-block chunks (full 1024-wide rows -> 4 KB descriptors).
  Image 0's first two chunks go on the two HWDGE rings (ready ~2 us before
  SWDGE's first byte); everything else streams on SWDGE with 4 image buffers
  so compute lag never throttles the stream. Pass-1 (f32r) pieces are emitted
  per chunk arrival; Y quarters evac to bf16, transpose on PE (identity
  matmul), then pass-2 (bf16 stationary from yt) accumulates Z rows.
  Consts are packed to the 36-wide band of T (tbz [128,8,36] bf16, ~74 KB)
  and expanded on-chip into a zeroed [128,8,256] tile; f32r copy cast on DVE.
  The last image ends with two 0.25 MB quarter-chunks (pc7/ch0, pc7/ch1) so
  only one pass-1 matmul + half of pass-2 trail the final byte, with evacs
  split across Vector+Scalar to halve chain latency.
Out is declared in SBUF layout [128, 6, 2, 256] and unscrambled on the host.
"""

import numpy as np
import ml_dtypes

import concourse.bacc as bacc
import concourse.mybir as mybir
import concourse.tile as tile
from concourse.bass_utils import run_bass_kernel_spmd

SIGMA = 0.66
BICUBIC_W = np.array([-0.09375, 0.59375, 0.59375, -0.09375], dtype=np.float64)

N_CORES = 8
B, C, H, W = 16, 3, 1024, 1024
HO = H // 4
IMGS = (B // N_CORES) * C  # 6 images per core
BAND = 36  # max nonzero column width of a 128-row block of Tt

F32 = mybir.dt.float32
F32R = mybir.dt.float32r
BF16 = mybir.dt.bfloat16


def _gauss_matrix(n):
    x = np.arange(3, dtype=np.float32) - np.float32(1.0)
    k = np.exp(np.float32(-0.5) * (x / np.float32(SIGMA)) ** 2)
    k = (k / k.sum()).astype(np.float64)
    G = np.zeros((n, n))
    for t in range(3):
        G += k[t] * np.eye(n, n, t - 1)
    return G


def _down_matrix(n):
    # out[i] = sum_t w[t] * x[clamp(2i + t - 1, 0, n-1)]
    m = n // 2
    D = np.zeros((m, n))
    for i in range(m):
        for t in range(4):
            j = min(max(2 * i + t - 1, 0), n - 1)
            D[i, j] += BICUBIC_W[t]
    return D


def build_T():
    T = _down_matrix(H // 2) @ _gauss_matrix(H // 2) @ _down_matrix(H) @ _gauss_matrix(H)
    return T.astype(np.float32)  # [256, 1024]


def _band_lo(Tt):
    """Per 128-row block of Tt, the left edge of its <=BAND nonzero columns."""
    lo = []
    for c in range(8):
        blk = Tt[128 * c : 128 * (c + 1)]
        nz = np.nonzero(np.any(blk != 0, axis=0))[0]
        assert nz.max() - nz.min() + 1 <= BAND
        lo.append(min(int(nz.min()), HO - BAND))
    return lo


PCS_BY_IH = [[0, 1, 2, 3, 4], [3, 4, 5, 6, 7]]


def _build_graph():
    Tt = build_T().T  # [1024, 256]
    band_lo = _band_lo(Tt)

    nc = bacc.Bacc("TRN2", target_bir_lowering=False, debug=False)
    # x viewed as [img, chunk(4), q(2), p(128), w] : row = 256*chunk + 128*q + p
    x = nc.dram_tensor("x", [IMGS, 4, 2, 128, W], F32R, kind="ExternalInput").ap()
    # packed consts: [0:288) = band of Tt (tbz[p, c*BAND+k] = Tt[128c+p,
    # band_lo[c]+k]), [288:416) = 128x128 identity. One DMA, 832 B descriptors.
    cst = nc.dram_tensor("cst", [128, 8 * BAND + 128], BF16,
                         kind="ExternalInput").ap()
    # out in SBUF layout [p, img, c, j] = Z[img, 128c+p, j]; host unscrambles
    out = nc.dram_tensor("out", [128, IMGS, 2, HO], F32, kind="ExternalOutput").ap()

    with tile.TileContext(nc) as tc:
        with (
            tc.tile_pool(name="const", bufs=1) as cpool,
            tc.tile_pool(name="xin", bufs=4) as xpool,
            tc.tile_pool(name="ysb", bufs=2) as ypool,
            tc.tile_pool(name="ytsb", bufs=2) as ytpool,
            tc.tile_pool(name="zquad", bufs=1) as zqpool,
            tc.tile_pool(name="zout", bufs=1) as zpool,
            tc.tile_pool(name="psy", bufs=4, space="PSUM") as psy,
            tc.tile_pool(name="pst", bufs=2, space="PSUM") as pst,
            tc.tile_pool(name="ps2", bufs=2, space="PSUM") as ps2,
        ):
            # image-0 chunks 0/1 on the two HWDGE rings: bytes moving
            # ~2 us before SWDGE's first byte, and nothing tiny ahead of them
            xt0 = xpool.tile([128, 8, W], F32R, tag="xt", name="xt0")
            nc.sync.dma_start(
                out=xt0[:, 0:2, :], in_=x[0, 0].rearrange("q p w -> p q w")
            )
            nc.scalar.dma_start(
                out=xt0[:, 2:4, :], in_=x[0, 1].rearrange("q p w -> p q w")
            )
            # consts ride the scalar ring AFTER the 1 MB chunk (tiny 832 B
            # descriptors would clog the HWDGE descriptor generator early)
            cst_sb = cpool.tile([128, 8 * BAND + 128], BF16, tag="cst")
            nc.scalar.dma_start(out=cst_sb[:], in_=cst)
            ident = cst_sb[:, 8 * BAND : 8 * BAND + 128]
            # expand packed band into zeroed dense [128, 8, 256]
            ttb = cpool.tile([128, 8, HO], BF16, tag="ttb")
            nc.vector.memset(ttb[:], 0.0)
            for c in range(8):
                nc.vector.tensor_copy(
                    ttb[:, c, band_lo[c] : band_lo[c] + BAND],
                    cst_sb[:, c * BAND : (c + 1) * BAND],
                )
            # f32r copy of Tt for pass-1 stationary
            tt = cpool.tile([128, 8, HO], F32R, tag="tt")
            nc.vector.tensor_copy(tt[:], ttb[:])

            def p1mm(yq, pc, ih, xap, start, stop):
                nc.tensor.matmul(
                    yq[:],
                    tt[:, pc, 128 * ih : 128 * (ih + 1)],
                    xap,
                    start=start,
                    stop=stop,
                )

            for img in range(IMGS):
                last = img == IMGS - 1
                xt = (xt0 if img == 0 else
                      xpool.tile([128, 8, W], F32R, tag="xt", name=f"xt{img}"))
                y_sb = ypool.tile([128, 2, W], BF16, tag="ysb", name=f"y{img}")
                yt_sb = ytpool.tile([128, 8, HO], BF16, tag="ytsb",
                                    name=f"yt{img}")
                if img == 0:
                    zq = zqpool.tile([128, 4, 2, HO], F32, tag="zq", name="zq")
                if img == 4:
                    zpair = zpool.tile([128, 2, 2, HO], F32, tag="zout",
                                       name="z4")
                z = zq[:, img] if img < 4 else zpair[:, img - 4]

                yq = {
                    (ch, ih): psy.tile(
                        [128, 512], F32, tag="psy", name=f"psy{img}_{ch}_{ih}"
                    )
                    for ch in range(2)
                    for ih in range(2)
                }
                accs = [
                    ps2.tile([128, HO], F32, tag="ps2", name=f"ps2_{img}_{ih}")
                    for ih in range(2)
                ]

                def load_chunk(c):
                    # rows 256c .. 256c+255, full width, 4 KB descriptors
                    if img == 0 and c < 2:
                        return  # already issued on the HWDGE rings
                    nc.gpsimd.dma_start(
                        out=xt[:, 2 * c : 2 * c + 2, :],
                        in_=x[img, c].rearrange("q p w -> p q w"),
                    )

                def chunk_mms(c):
                    # pass-1 pieces unlocked by chunk c (pcs 2c, 2c+1)
                    for ih in range(2):
                        pcs = PCS_BY_IH[ih]
                        for pc in (2 * c, 2 * c + 1):
                            if pc not in pcs:
                                continue
                            for ch in range(2):
                                p1mm(
                                    yq[ch, ih],
                                    pc,
                                    ih,
                                    xt[:, pc, 512 * ch : 512 * (ch + 1)],
                                    start=pc == pcs[0],
                                    stop=pc == pcs[-1],
                                )

                def evac_y(ch, ih, split):
                    dst = y_sb[:, ih, 512 * ch : 512 * (ch + 1)]
                    if split:
                        nc.vector.tensor_copy(dst[:, 0:256], yq[ch, ih][:, 0:256])
                        nc.scalar.copy(dst[:, 256:512], yq[ch, ih][:, 256:512])
                    elif ch == 0:
                        nc.vector.tensor_copy(dst, yq[ch, ih][:])
                    else:
                        nc.scalar.copy(dst, yq[ch, ih][:])

                def transposes(ch, ih, split):
                    tp = pst.tile(
                        [128, 512], BF16, tag="pst", name=f"tp{img}_{ch}_{ih}"
                    )
                    for s in range(4):
                        qc = 4 * ch + s
                        nc.tensor.matmul(
                            tp[:, 128 * s : 128 * (s + 1)],
                            y_sb[:, ih, 128 * qc : 128 * (qc + 1)],
                            ident,
                            is_transpose=True,
                            start=(s == 0),
                            stop=(s == 3),
                        )
                    dst = yt_sb[:, 4 * ch : 4 * ch + 4, 128 * ih : 128 * (ih + 1)]
                    tsrc = tp[:].rearrange("p (s w) -> p s w", s=4)
                    if split:
                        nc.vector.tensor_copy(dst[:, 0:2], tsrc[:, 0:2])
                        nc.scalar.copy(dst[:, 2:4], tsrc[:, 2:4])
                    elif ch == 0:
                        nc.vector.tensor_copy(dst, tsrc)
                    else:
                        nc.scalar.copy(dst, tsrc)

                def p2mm(ih, qcs, start, stop):
                    for qc in qcs:
                        nc.tensor.matmul(
                            accs[ih][:],
                            yt_sb[:, qc, 128 * ih : 128 * (ih + 1)],
                            ttb[:, qc, :],
                            start=start and qc == qcs[0],
                            stop=stop and qc == qcs[-1],
                        )

                def evac_z(ih, split):
                    if split:
                        nc.vector.tensor_copy(z[:, ih, 0:128], accs[ih][:, 0:128])
                        nc.scalar.copy(z[:, ih, 128:256], accs[ih][:, 128:256])
                    elif ih == 0:
                        nc.vector.tensor_copy(z[:, ih, :], accs[ih][:])
                    else:
                        nc.scalar.copy(z[:, ih, :], accs[ih][:])

                if not last:
                    for c in range(4):
                        load_chunk(c)
                    chunk_mms(0)
                    chunk_mms(1)
                    # chunk 2 (pcs 4,5): completes ih0
                    chunk_mms(2)
                    for ch in range(2):
                        evac_y(ch, 0, split=False)
                    for ch in range(2):
                        transposes(ch, 0, split=False)
                    # chunk 3 (pcs 6,7): completes ih1
                    chunk_mms(3)
                    p2mm(0, range(8), start=True, stop=True)
                    for ch in range(2):
                        evac_y(ch, 1, split=False)
                    evac_z(0, split=False)
                    for ch in range(2):
                        transposes(ch, 1, split=False)
                    p2mm(1, range(8), start=True, stop=True)
                    evac_z(1, split=False)
                    if img == 3:
                        # one store for images 0-3: 8 KB descriptors, and
                        # fewer store ops polluting the DMA sem-lane
                        # rotation that gates load descriptor generation
                        nc.sync.dma_start(out=out[:, 0:4], in_=zq[:])
                    elif img == IMGS - 2:
                        # penultimate image stores alone, overlapping the
                        # last image's compute
                        nc.sync.dma_start(out=out[:, img], in_=z[:])
                else:
                    # last image: 1 MB chunks 0-2, then pc6/pc7 as four
                    # 0.25 MB quarters ordered ch0-first, so the ch0 half of
                    # the finishing chain overlaps the last two transfers and
                    # only the ch1 half trails the final byte.
                    for c in range(3):
                        load_chunk(c)
                    for ch in range(2):
                        for pc in (6, 7):
                            nc.gpsimd.dma_start(
                                out=xt[:, pc, 512 * ch : 512 * (ch + 1)],
                                in_=x[img, 3, pc - 6, :,
                                      512 * ch : 512 * (ch + 1)],
                            )
                    chunk_mms(0)
                    chunk_mms(1)
                    chunk_mms(2)
                    for ch in range(2):
                        evac_y(ch, 0, split=False)
                    for ch in range(2):
                        transposes(ch, 0, split=False)
                    p2mm(0, range(8), start=True, stop=True)
                    evac_z(0, split=True)
                    nc.sync.dma_start(out=out[:, img, 0], in_=z[:, 0, :])
                    # ch0 side of ih1 finishes while ch1 still streams
                    p1mm(yq[0, 1], 6, 1, xt[:, 6, 0:512], False, False)
                    p1mm(yq[0, 1], 7, 1, xt[:, 7, 0:512], False, True)
                    p1mm(yq[1, 1], 6, 1, xt[:, 6, 512:1024], False, False)
                    evac_y(0, 1, split=True)
                    transposes(0, 1, split=True)
                    p2mm(1, range(4), start=True, stop=False)
                    # pc7 ch1 quarter -> finish everything
                    p1mm(yq[1, 1], 7, 1, xt[:, 7, 512:1024], False, True)
                    evac_y(1, 1, split=True)
                    transposes(1, 1, split=True)
                    p2mm(1, range(4, 8), start=False, stop=True)
                    evac_z(1, split=True)
                    nc.sync.dma_start(out=out[:, img, 1], in_=z[:, 1, :])
    nc.compile()
    return nc


_GRAPH = None


def _get_graph():
    global _GRAPH
    if _GRAPH is None:
        _GRAPH = _build_graph()
    return _GRAPH


def run(x, **spmd_kwargs):
    x = np.ascontiguousarray(np.asarray(x, dtype=np.float32))
    assert x.shape == (B, C, H, W)
    nc = _get_graph()
    Tt = build_T().T  # [1024, 256] f32
    band_lo = _band_lo(Tt)
    cst_host = np.zeros((128, 8 * BAND + 128), dtype=np.float32)
    for c in range(8):
        cst_host[:, c * BAND : (c + 1) * BAND] = Tt[
            128 * c : 128 * (c + 1), band_lo[c] : band_lo[c] + BAND
        ]
    cst_host[:, 8 * BAND :] = np.eye(128, dtype=np.float32)
    cst_host = cst_host.astype(ml_dtypes.bfloat16)
    per_core = B // N_CORES
    in_maps = [
        {
            "x": x[i * per_core : (i + 1) * per_core].reshape(IMGS, 4, 2, 128, W),
            "cst": cst_host,
        }
        for i in range(N_CORES)
    ]
    res = run_bass_kernel_spmd(nc, in_maps, core_ids=list(range(N_CORES)), **spmd_kwargs)
    outs = []
    for r in res.results:
        o = r["out"].transpose(1, 2, 0, 3).reshape(IMGS, 2 * 128, HO)
        outs.append(o.reshape(per_core, C, HO, HO))
    return np.concatenate(outs, axis=0), res


def kernel(x):
    out, _ = run(x)
    return out


# revision 16
# speedup vs baseline: 1.1119x; 1.1119x over previous
"""Antialiased bicubic 4x downscale (blur -> bicubic/2, twice) on 8 TRN2 cores.

The whole chain is linear and separable: every stage is M_H (x) M_W acting on
the H/W axes, so the composition collapses to a single 1024->256 banded matrix
T applied on both sides: out = T @ X @ T^T per (batch, channel) image.

Sharding: pure data parallel - batch 16 -> 2 per core, 6 images/core.

Schedule (v3): the measured window starts right after the engine barrier and
ends after the fixed semaphore-sweep postamble; SWDGE with full-row 4 KB
descriptors sustains ~400 GB/s, so the kernel keeps one deep SWDGE stream and
gets everything else out of its way.

  x arrives in row-block chunks (full 1024-wide rows -> 4 KB descriptors).
  Image 0's first two chunks go on the two HWDGE rings (ready ~2 us before
  SWDGE's first byte); everything else streams on SWDGE with 4 image buffers
  so compute lag never throttles the stream. Pass-1 (f32r) pieces are emitted
  per chunk arrival; Y quarters evac to bf16, transpose on PE (identity
  matmul), then pass-2 (bf16 stationary from yt) accumulates Z rows.
  Consts are packed to the 36-wide band of T (tbz [128,8,36] bf16, ~74 KB)
  and expanded on-chip into a zeroed [128,8,256] tile; f32r copy cast on DVE.
  The last image ends with two 0.25 MB quarter-chunks (pc7/ch0, pc7/ch1) so
  only one pass-1 matmul + half of pass-2 trail the final byte, with evacs
  split across Vector+Scalar to halve chain latency.
Out is declared in SBUF layout [128, 6, 2, 256] and unscrambled on the host.
"""

import numpy as np
import ml_dtypes

import concourse.bacc as bacc
import concourse.mybir as mybir
import concourse.tile as tile
from concourse.bass_utils import run_bass_kernel_spmd

SIGMA = 0.66
BICUBIC_W = np.array([-0.09375, 0.59375, 0.59375, -0.09375], dtype=np.float64)

N_CORES = 8
B, C, H, W = 16, 3, 1024, 1024
HO = H // 4
IMGS = (B // N_CORES) * C  # 6 images per core
BAND = 36  # max nonzero column width of a 128-row block of Tt

F32 = mybir.dt.float32
F32R = mybir.dt.float32r
BF16 = mybir.dt.bfloat16


def _gauss_matrix(n):
    x = np.arange(3, dtype=np.float32) - np.float32(1.0)
    k = np.exp(np.float32(-0.5) * (x / np.float32(SIGMA)) ** 2)
    k = (k / k.sum()).astype(np.float64)
    G = np.zeros((n, n))
    for t in range(3):
        G += k[t] * np.eye(n, n, t - 1)
    return G


def _down_matrix(n):
    # out[i] = sum_t w[t] * x[clamp(2i + t - 1, 0, n-1)]
    m = n // 2
    D = np.zeros((m, n))
    for i in range(m):
        for t in range(4):
            j = min(max(2 * i + t - 1, 0), n - 1)
            D[i, j] += BICUBIC_W[t]
    return D


def build_T():
    T = _down_matrix(H // 2) @ _gauss_matrix(H // 2) @ _down_matrix(H) @ _gauss_matrix(H)
    return T.astype(np.float32)  # [256, 1024]


def _band_lo(Tt):
    """Per 128-row block of Tt, the left edge of its <=BAND nonzero columns."""
    lo = []
    for c in range(8):
        blk = Tt[128 * c : 128 * (c + 1)]
        nz = np.nonzero(np.any(blk != 0, axis=0))[0]
        assert nz.max() - nz.min() + 1 <= BAND
        lo.append(min(int(nz.min()), HO - BAND))
    return lo


PCS_BY_IH = [[0, 1, 2, 3, 4], [3, 4, 5, 6, 7]]


def _build_graph():
    Tt = build_T().T  # [1024, 256]
    band_lo = _band_lo(Tt)

    nc = bacc.Bacc("TRN2", target_bir_lowering=False, debug=False)
    # x viewed as [img, chunk(4), q(2), p(128), w] : row = 256*chunk + 128*q + p
    x = nc.dram_tensor("x", [IMGS, 4, 2, 128, W], F32R, kind="ExternalInput").ap()
    # packed consts: [0:288) = band of Tt (tbz[p, c*BAND+k] = Tt[128c+p,
    # band_lo[c]+k]), [288:416) = 128x128 identity. One DMA, 832 B descriptors.
    cst = nc.dram_tensor("cst", [128, 8 * BAND + 128], BF16,
                         kind="ExternalInput").ap()
    # out in SBUF layout [p, img, c, j] = Z[img, 128c+p, j]; host unscrambles
    out = nc.dram_tensor("out", [128, IMGS, 2, HO], F32, kind="ExternalOutput").ap()

    with tile.TileContext(nc) as tc:
        with (
            tc.tile_pool(name="const", bufs=1) as cpool,
            tc.tile_pool(name="xin", bufs=4) as xpool,
            tc.tile_pool(name="ysb", bufs=2) as ypool,
            tc.tile_pool(name="ytsb", bufs=2) as ytpool,
            tc.tile_pool(name="zout", bufs=2) as zpool,
            tc.tile_pool(name="psy", bufs=4, space="PSUM") as psy,
            tc.tile_pool(name="pst", bufs=2, space="PSUM") as pst,
            tc.tile_pool(name="ps2", bufs=2, space="PSUM") as ps2,
        ):
            # image-0 chunks 0/1 on the two HWDGE rings: bytes moving
            # ~2 us before SWDGE's first byte, and nothing tiny ahead of them
            xt0 = xpool.tile([128, 8, W], F32R, tag="xt", name="xt0")
            nc.sync.dma_start(
                out=xt0[:, 0:2, :], in_=x[0, 0].rearrange("q p w -> p q w")
            )
            nc.scalar.dma_start(
                out=xt0[:, 2:4, :], in_=x[0, 1].rearrange("q p w -> p q w")
            )
            # consts ride the scalar ring AFTER the 1 MB chunk (tiny 832 B
            # descriptors would clog the HWDGE descriptor generator early)
            cst_sb = cpool.tile([128, 8 * BAND + 128], BF16, tag="cst")
            nc.scalar.dma_start(out=cst_sb[:], in_=cst)
            ident = cst_sb[:, 8 * BAND : 8 * BAND + 128]
            # expand packed band into zeroed dense [128, 8, 256]
            ttb = cpool.tile([128, 8, HO], BF16, tag="ttb")
            nc.vector.memset(ttb[:], 0.0)
            for c in range(8):
                nc.vector.tensor_copy(
                    ttb[:, c, band_lo[c] : band_lo[c] + BAND],
                    cst_sb[:, c * BAND : (c + 1) * BAND],
                )
            # f32r copy of Tt for pass-1 stationary
            tt = cpool.tile([128, 8, HO], F32R, tag="tt")
            nc.vector.tensor_copy(tt[:], ttb[:])

            def p1mm(yq, pc, ih, xap, start, stop):
                nc.tensor.matmul(
                    yq[:],
                    tt[:, pc, 128 * ih : 128 * (ih + 1)],
                    xap,
                    start=start,
                    stop=stop,
                )

            for img in range(IMGS):
                last = img == IMGS - 1
                xt = (xt0 if img == 0 else
                      xpool.tile([128, 8, W], F32R, tag="xt", name=f"xt{img}"))
                y_sb = ypool.tile([128, 2, W], BF16, tag="ysb", name=f"y{img}")
                yt_sb = ytpool.tile([128, 8, HO], BF16, tag="ytsb",
                                    name=f"yt{img}")
                if img % 2 == 0:
                    zpair = zpool.tile(
                        [128, 2, 2, HO], F32, tag="zout", name=f"z{img}"
                    )
                z = zpair[:, img % 2]

                yq = {
                    (ch, ih): psy.tile(
                        [128, 512], F32, tag="psy", name=f"psy{img}_{ch}_{ih}"
                    )
                    for ch in range(2)
                    for ih in range(2)
                }
                accs = [
                    ps2.tile([128, HO], F32, tag="ps2", name=f"ps2_{img}_{ih}")
                    for ih in range(2)
                ]

                def load_chunk(c):
                    # rows 256c .. 256c+255, full width, 4 KB descriptors
                    if img == 0 and c < 2:
                        return  # already issued on the HWDGE rings
                    nc.gpsimd.dma_start(
                        out=xt[:, 2 * c : 2 * c + 2, :],
                        in_=x[img, c].rearrange("q p w -> p q w"),
                    )

                def chunk_mms(c):
                    # pass-1 pieces unlocked by chunk c (pcs 2c, 2c+1)
                    for ih in range(2):
                        pcs = PCS_BY_IH[ih]
                        for pc in (2 * c, 2 * c + 1):
                            if pc not in pcs:
                                continue
                            for ch in range(2):
                                p1mm(
                                    yq[ch, ih],
                                    pc,
                                    ih,
                                    xt[:, pc, 512 * ch : 512 * (ch + 1)],
                                    start=pc == pcs[0],
                                    stop=pc == pcs[-1],
                                )

                def evac_y(ch, ih, split):
                    dst = y_sb[:, ih, 512 * ch : 512 * (ch + 1)]
                    if split:
                        nc.vector.tensor_copy(dst[:, 0:256], yq[ch, ih][:, 0:256])
                        nc.scalar.copy(dst[:, 256:512], yq[ch, ih][:, 256:512])
                    elif ch == 0:
                        nc.vector.tensor_copy(dst, yq[ch, ih][:])
                    else:
                        nc.scalar.copy(dst, yq[ch, ih][:])

                def transposes(ch, ih, split):
                    tp = pst.tile(
                        [128, 512], BF16, tag="pst", name=f"tp{img}_{ch}_{ih}"
                    )
                    for s in range(4):
                        qc = 4 * ch + s
                        nc.tensor.matmul(
                            tp[:, 128 * s : 128 * (s + 1)],
                            y_sb[:, ih, 128 * qc : 128 * (qc + 1)],
                            ident,
                            is_transpose=True,
                            start=(s == 0),
                            stop=(s == 3),
                        )
                    dst = yt_sb[:, 4 * ch : 4 * ch + 4, 128 * ih : 128 * (ih + 1)]
                    tsrc = tp[:].rearrange("p (s w) -> p s w", s=4)
                    if split:
                        nc.vector.tensor_copy(dst[:, 0:2], tsrc[:, 0:2])
                        nc.scalar.copy(dst[:, 2:4], tsrc[:, 2:4])
                    elif ch == 0:
                        nc.vector.tensor_copy(dst, tsrc)
                    else:
                        nc.scalar.copy(dst, tsrc)

                def p2mm(ih, qcs, start, stop):
                    for qc in qcs:
                        nc.tensor.matmul(
                            accs[ih][:],
                            yt_sb[:, qc, 128 * ih : 128 * (ih + 1)],
                            ttb[:, qc, :],
                            start=start and qc == qcs[0],
                            stop=stop and qc == qcs[-1],
                        )

                def evac_z(ih, split):
                    if split:
                        nc.vector.tensor_copy(z[:, ih, 0:128], accs[ih][:, 0:128])
                        nc.scalar.copy(z[:, ih, 128:256], accs[ih][:, 128:256])
                    elif ih == 0:
                        nc.vector.tensor_copy(z[:, ih, :], accs[ih][:])
                    else:
                        nc.scalar.copy(z[:, ih, :], accs[ih][:])

                if not last:
                    for c in range(4):
                        load_chunk(c)
                    chunk_mms(0)
                    chunk_mms(1)
                    # chunk 2 (pcs 4,5): completes ih0
                    chunk_mms(2)
                    for ch in range(2):
                        evac_y(ch, 0, split=False)
                    for ch in range(2):
                        transposes(ch, 0, split=False)
                    # chunk 3 (pcs 6,7): completes ih1
                    chunk_mms(3)
                    p2mm(0, range(8), start=True, stop=True)
                    for ch in range(2):
                        evac_y(ch, 1, split=False)
                    evac_z(0, split=False)
                    for ch in range(2):
                        transposes(ch, 1, split=False)
                    p2mm(1, range(8), start=True, stop=True)
                    evac_z(1, split=False)
                    if img == IMGS - 2:
                        # penultimate image stores alone, overlapping the
                        # last image's compute
                        nc.sync.dma_start(out=out[:, img], in_=z[:])
                    elif img % 2 == 1:
                        nc.sync.dma_start(
                            out=out[:, img - 1 : img + 1], in_=zpair[:]
                        )
                else:
                    # last image: 1 MB chunks 0-2, then pc6/pc7 as four
                    # 0.25 MB quarters ordered ch0-first, so the ch0 half of
                    # the finishing chain overlaps the last two transfers and
                    # only the ch1 half trails the final byte.
                    for c in range(3):
                        load_chunk(c)
                    for ch in range(2):
                        for pc in (6, 7):
                            nc.gpsimd.dma_start(
                                out=xt[:, pc, 512 * ch : 512 * (ch + 1)],
                                in_=x[img, 3, pc - 6, :,
                                      512 * ch : 512 * (ch + 1)],
                            )
                    chunk_mms(0)
                    chunk_mms(1)
                    chunk_mms(2)
                    for ch in range(2):
                        evac_y(ch, 0, split=False)
                    for ch in range(2):
                        transposes(ch, 0, split=False)
                    p2mm(0, range(8), start=True, stop=True)
                    evac_z(0, split=True)
                    nc.sync.dma_start(out=out[:, img, 0], in_=z[:, 0, :])
                    # ch0 side of ih1 finishes while ch1 still streams
                    p1mm(yq[0, 1], 6, 1, xt[:, 6, 0:512], False, False)
                    p1mm(yq[0, 1], 7, 1, xt[:, 7, 0:512], False, True)
                    p1mm(yq[1, 1], 6, 1, xt[:, 6, 512:1024], False, False)
                    evac_y(0, 1, split=True)
                    transposes(0, 1, split=True)
                    p2mm(1, range(4), start=True, stop=False)
                    # pc7 ch1 quarter -> finish everything
                    p1mm(yq[1, 1], 7, 1, xt[:, 7, 512:1024], False, True)
                    evac_y(1, 1, split=True)
                    transposes(1, 1, split=True)
                    p2mm(1, range(4, 8), start=False, stop=True)
                    evac_z(1, split=True)
                    nc.sync.dma_start(out=out[:, img, 1], in_=z[:, 1, :])
    nc.compile()
    return nc


_GRAPH = None


def _get_graph():
    global _GRAPH
    if _GRAPH is None:
        _GRAPH = _build_graph()
    return _GRAPH


def run(x, **spmd_kwargs):
    x = np.ascontiguousarray(np.asarray(x, dtype=np.float32))
    assert x.shape == (B, C, H, W)
    nc = _get_graph()
    Tt = build_T().T  # [1024, 256] f32
    band_lo = _band_lo(Tt)
    cst_host = np.zeros((128, 8 * BAND + 128), dtype=np.float32)
    for c in range(8):
        cst_host[:, c * BAND : (c + 1) * BAND] = Tt[
            128 * c : 128 * (c + 1), band_lo[c] : band_lo[c] + BAND
        ]
    cst_host[:, 8 * BAND :] = np.eye(128, dtype=np.float32)
    cst_host = cst_host.astype(ml_dtypes.bfloat16)
    per_core = B // N_CORES
    in_maps = [
        {
            "x": x[i * per_core : (i + 1) * per_core].reshape(IMGS, 4, 2, 128, W),
            "cst": cst_host,
        }
        for i in range(N_CORES)
    ]
    res = run_bass_kernel_spmd(nc, in_maps, core_ids=list(range(N_CORES)), **spmd_kwargs)
    outs = []
    for r in res.results:
        o = r["out"].transpose(1, 2, 0, 3).reshape(IMGS, 2 * 128, HO)
        outs.append(o.reshape(per_core, C, HO, HO))
    return np.concatenate(outs, axis=0), res


def kernel(x):
    out, _ = run(x)
    return out
